# revision 39
# baseline (speedup 1.0000x reference)
"""AdaptiveGraphConv (Chebyshev K=3) Trainium2 kernel, 8-core data-parallel.

Math (per (batch,time) item, x_item [N,C]):
  M = D^-1/2 A D^-1/2  (normalized adjacency; L = I - M), M symmetric.
  T0 = x; T1 = Lx; T2 = 2L T1 - T0
  out = T0 W0 + T1 W1 + T2 W2 + b
      = x (W0+W1+W2) + (Mx)(-W1-4W2) + (M^2 x)(2W2) + b
M^2 is precomputed once (325x325), so both node-contractions read the same
node-major x and write channel-major results directly (no back-transposes):
  MX_cm[(b,c), i] = sum_j X_nm[j, (b,c)] * M[j, i]   (X_nm as stationary)
Sharding: data-parallel over batch dim B=64 -> 8 batches/core. Laplacian,
weights, bias replicated. No collectives.

Schedule notes (v6):
 - ~18 dense 512-col dummy matmuls at boot keep the PE HAM activity window
   busy (2.4GHz, not the cold 1.2GHz) until the first real matmuls.
 - pair-0 inputs split across both HWDGE rings: sync carries adj + x
   chunks 0/2, the scalar ring carries x chunk 1 (+ W, bias). 128-
   partition (b c)-folded transfers throughout.
 - f32->bf16 convert (strided read reorder (n,t)->(t,n)) split
   vector(c0) / scalar(c1) / vector(c2); PE transposes run in chunk
   order 0,2,1 so the slowest convert (scalar c1) has the most headroom.
 - x loads for pair p+1 issue at the START of pair p (converts+transposes
   emitted between M-apply and W-stage) so the pair boundary never waits
   on DMA.
 - transpose-psum and W-stage-psum share one 4-buf pool: the W-stage gets
   a 4-deep PSUM rotation instead of 2.
 - outputs stream out in 2-block chunks as soon as evicted (sync/scalar
   rings); the last pair goes per-block so the tail is one small DMA.
"""
import os
import sys
import numpy as np

_TRN_REPO = "/opt/trn_rl_repo"
if _TRN_REPO not in sys.path:
    sys.path.insert(0, _TRN_REPO)


def _ensure_ntff_hook():
    """Make antenv.axon_hooks importable so NTFF profiling can register."""
    src = (
        "_hook = None\n"
        "def set_axon_ntff_profile_hook(hook):\n"
        "    global _hook\n"
        "    _hook = hook\n"
        "def get_axon_ntff_profile_hook():\n"
        "    return _hook\n"
    )
    try:
        import antenv  # noqa
        base = os.path.dirname(antenv.__file__)
        path = os.path.join(base, "axon_hooks.py")
        if not os.path.exists(path):
            with open(path, "w") as f:
                f.write(src)
    except Exception:
        pass


_ensure_ntff_hook()

B, C, N, T, K = 64, 64, 325, 12, 3
NCORES = 8
B_LOC = B // NCORES          # 8 batches per core
NPAIRS = B_LOC // 2          # 4 pairs of batches
NT = N * T                   # 3900
CNT = [128, 128, 69]         # node chunk sizes (325 = 128+128+69)
NOFF = [0, 128, 256]
NBLK = 42                    # W-stage node-block (504 cols <= one PSUM bank)

_cache = {}


def _build():
    import concourse.bass as bass  # noqa
    import concourse.bacc as bacc
    import concourse.mybir as mybir
    import concourse.tile as tile
    from concourse import masks
    from contextlib import ExitStack

    f32 = mybir.dt.float32
    bf16 = mybir.dt.bfloat16
    ALU = mybir.AluOpType
    AF = mybir.ActivationFunctionType

    nc = bacc.Bacc("TRN2", target_bir_lowering=False, debug=False,
                   num_devices=NCORES)
    x_ext = nc.dram_tensor("x", [B_LOC, C, N, T], f32, kind="ExternalInput")
    adj_ext = nc.dram_tensor("adj", [N, N], f32, kind="ExternalInput")
    w_ext = nc.dram_tensor("W", [K, C, C], f32, kind="ExternalInput")
    b_ext = nc.dram_tensor("b", [C], f32, kind="ExternalInput")
    out_ext = nc.dram_tensor("out", [B_LOC, C, N, T], f32,
                             kind="ExternalOutput")

    with tile.TileContext(nc) as tc, ExitStack() as ctx:
        const = ctx.enter_context(tc.tile_pool(name="const", bufs=1))
        # M-apply psum: two 2-bank tiles (MX+M2X for one t each, t and
        # t+1 in flight; the single merged eviction overlaps t+1's MMs)
        ps_m = ctx.enter_context(
            tc.tile_pool(name="ps_m", bufs=2, space="PSUM"))
        # shared transpose/W-stage psum rotation: 4 single-bank bufs
        ps_tw = ctx.enter_context(
            tc.tile_pool(name="ps_tw", bufs=4, space="PSUM"))

        xs_pool = ctx.enter_context(tc.tile_pool(name="xs", bufs=2))
        nm_pool = ctx.enter_context(tc.tile_pool(name="nm", bufs=2))
        cm_pool = ctx.enter_context(tc.tile_pool(name="cm", bufs=2))
        out_pool = ctx.enter_context(tc.tile_pool(name="outp", bufs=2))

        state = {}

        def emit_loads(p, split=False):
            # chunked so the per-chunk convert starts as each chunk lands;
            # both batch-halves ride one 128-partition DMA (b,c fold).
            # split=True routes chunk 1 over the scalar ring (pair 0: both
            # rings pull x concurrently).
            Xf = xs_pool.tile([128, N, T], f32, tag="xf", name="xf")
            xsrc = x_ext.ap()[2 * p: 2 * p + 2].rearrange(
                "b c n t -> (b c) n t")
            for i in range(3):
                nsl = slice(NOFF[i], NOFF[i] + CNT[i])
                eng = nc.scalar if (split and i >= 1) else nc.sync
                eng.dma_start(Xf[:, nsl, :], xsrc[:, nsl, :])
            state[p] = Xf

        def emit_convert(p):
            # f32 (n,t) -> bf16 (t,n): the reorder rides on the strided
            # READ (strided reads are cheap; strided writes are not).
            # chunk 0 -> vector, chunk 1 -> split scalar/vector (it's the
            # big slow one), chunk 2 -> vector.
            Xf = state.pop(p)
            Xs = xs_pool.tile([128, T, N], bf16, tag="xsb", name="xsb")
            for i in range(3):
                nsl = slice(NOFF[i], NOFF[i] + CNT[i])
                if i == 1:
                    s0 = Xf[:, nsl, 0:6].rearrange("p n t -> p t n")
                    s1 = Xf[:, nsl, 6:12].rearrange("p n t -> p t n")
                    nc.scalar.activation(Xs[:, 0:6, nsl], s0, AF.Copy)
                    nc.vector.tensor_copy(Xs[:, 6:12, nsl], s1)
                else:
                    srcv = Xf[:, nsl, :].rearrange("p n t -> p t n")
                    nc.vector.tensor_copy(Xs[:, :, nsl], srcv)
            state[p] = Xs

        def emit_transposes(p):
            # node-major X via PE transposes: 3 tiles [n<=128, T, 128],
            # chunk order 0,1,2 matching convert arrival.
            Xs = state[p]
            XN = [nm_pool.tile([128, T, 128], bf16, tag=f"xn{i}",
                               name=f"xn{i}") for i in range(3)]
            for i in range(3):
                nsl = slice(NOFF[i], NOFF[i] + CNT[i])
                for tg in range(2):
                    ps = ps_tw.tile([128, 6, 128], bf16, tag="pstw",
                                    name="pst")
                    for tt in range(6):
                        t = tg * 6 + tt
                        nc.tensor.matmul(
                            ps[: CNT[i], tt, :],
                            Xs[:, t, nsl],
                            idn[:], is_transpose=True)
                    if tg == 0:
                        nc.scalar.activation(
                            XN[i][: CNT[i], 0:6, :],
                            ps[: CNT[i], :, :], AF.Copy)
                    else:
                        nc.vector.tensor_copy(
                            XN[i][: CNT[i], 6:12, :],
                            ps[: CNT[i], :, :])
            state[p] = (Xs, XN)

        # identity + PE warmup: dense 512-col dummy matmuls keep the PE
        # array at high duty so the HAM clock gate opens (2.4GHz) and
        # stays open until the first real matmuls. Results never read.
        idn = const.tile([128, 128], bf16)
        masks.make_identity(nc, idn[:])
        ztile = const.tile([128, 512], bf16)
        nc.vector.memset(ztile[:], 0.0)
        wps = ps_tw.tile([128, 512], f32, tag="pstw", name="wps")
        for _ in range(16):
            nc.tensor.matmul(wps[:, :], idn[:], ztile[:],
                             start=True, stop=True)

        # adj + x split across both rings (see emit_loads)
        Af = [const.tile([128, N], f32, tag=f"a{i}", name=f"a{i}")
              for i in range(3)]
        for i in range(3):
            nc.sync.dma_start(Af[i][: CNT[i], :],
                              adj_ext.ap()[NOFF[i]: NOFF[i] + CNT[i], :])
        emit_loads(0, split=True)

        # ---- M = D^-1/2 A D^-1/2, three node-row tiles [cnt, 325] bf16 ----
        s_col = [const.tile([128, 1], f32, tag=f"s{i}", name=f"s{i}")
                 for i in range(3)]
        for i in range(3):
            d = const.tile([128, 1], f32, tag="dtmp")
            nc.vector.reduce_sum(d[: CNT[i], :], Af[i][: CNT[i], :],
                                 axis=mybir.AxisListType.X)
            nc.scalar.activation(d[: CNT[i], :], d[: CNT[i], :], AF.Sqrt)
            nc.vector.reciprocal(s_col[i][: CNT[i], :], d[: CNT[i], :])
        # s as a row vector [1, N] via tiny transposes (f32 path)
        idf = const.tile([128, 128], f32)
        masks.make_identity(nc, idf[:])
        ps_s = ps_tw.tile([1, 512], f32, tag="pstw", name="ps_s")
        for i in range(3):
            nc.tensor.matmul(ps_s[0:1, NOFF[i]: NOFF[i] + CNT[i]],
                             s_col[i][: CNT[i], 0:1], idf[: CNT[i], : CNT[i]],
                             is_transpose=True)
        # broadcast s_row to 128 partitions via a bf16 matmul (s only
        # carries bf16 precision anyway: M itself is bf16)
        s_row = const.tile([1, N], bf16)
        nc.vector.tensor_copy(s_row[:], ps_s[0:1, :N])
        ones = const.tile([1, 128], bf16)
        nc.vector.memset(ones[:], 1.0)
        ps_b = ps_tw.tile([128, 512], f32, tag="pstw", name="ps_b")
        nc.tensor.matmul(ps_b[:, :N], ones[0:1, :], s_row[0:1, :])
        s_bc = const.tile([128, N], f32)
        nc.vector.tensor_copy(s_bc[:], ps_b[:, :N])
        # M_i = (s_col * A * s_row) -> bf16
        M = [const.tile([128, N], bf16, tag=f"m{i}", name=f"m{i}")
             for i in range(3)]
        for i in range(3):
            nc.vector.tensor_mul(Af[i][: CNT[i], :], Af[i][: CNT[i], :],
                                 s_bc[: CNT[i], :])
            nc.vector.tensor_scalar_mul(M[i][: CNT[i], :], Af[i][: CNT[i], :],
                                        s_col[i][: CNT[i], 0:1])
        # pair-0 convert AFTER the full M setup: its casts would otherwise
        # block the chain's vector ops (recip, muls) in the vector FIFO
        emit_convert(0)

        # ---- M2 = M @ M, three node-row tiles [cnt, 325] bf16 ----
        M2 = [const.tile([128, N], bf16, tag=f"m2_{i}", name=f"m2_{i}")
              for i in range(3)]
        for j in range(3):
            ps = ps_tw.tile([128, 512], f32, tag="pstw", name="psm2")
            for k in range(3):
                nc.tensor.matmul(
                    ps[: CNT[j], :N],
                    M[k][: CNT[k], NOFF[j]: NOFF[j] + CNT[j]],
                    M[k][: CNT[k], :],
                    start=(k == 0), stop=(k == 2))
            nc.scalar.activation(M2[j][: CNT[j], :], ps[: CNT[j], :N],
                                 AF.Copy)

        # ---- weight combos as block-diagonal [128,128] bf16 (2 copies) ----
        # Wa = W0+W1+W2 ; Wb = -W1-4W2 ; Wc = 2W2
        Wsb = const.tile([128, K, C], f32)
        for h in (0, 1):
            nc.scalar.dma_start(Wsb[64 * h: 64 * h + 64, :, :],
                                w_ext.ap().rearrange("k c d -> c k d"))
        Wa = const.tile([128, 128], bf16)
        Wb = const.tile([128, 128], bf16)
        Wc = const.tile([128, 128], bf16)
        for wt in (Wa, Wb, Wc):
            nc.gpsimd.memset(wt[:], 0.0)
        Wtmp = const.tile([128, C], f32)
        for h in (0, 1):
            r = slice(64 * h, 64 * h + 64)
            # Wa = (W0 + W1) + W2
            nc.vector.tensor_add(Wtmp[r, :], Wsb[r, 0, :], Wsb[r, 1, :])
            nc.vector.tensor_add(Wa[r, r], Wtmp[r, :], Wsb[r, 2, :])
            # Wb = (W2 * -4) - W1
            nc.vector.scalar_tensor_tensor(Wb[r, r], Wsb[r, 2, :], -4.0,
                                           Wsb[r, 1, :], ALU.mult,
                                           ALU.subtract)
            # Wc = 2*W2
            nc.vector.tensor_scalar_mul(Wc[r, r], Wsb[r, 2, :], 2.0)

        bias = const.tile([128, 1], f32)
        for h in (0, 1):
            nc.scalar.dma_start(bias[64 * h: 64 * h + 64, :], b_ext.ap())

        emit_transposes(0)
        for p in range(NPAIRS):
            Xs, XN = state.pop(p)

            # next pair's x load issues now so its convert never waits
            if p + 1 < NPAIRS:
                emit_loads(p + 1)

            # M-apply: MX and M2X in one channel-major, t-major tile
            # MXX[:, 0] = MX, MXX[:, 1] = M2X. Both land in one 2-bank
            # psum tile per t and leave in a single eviction.
            MXX = cm_pool.tile([128, 2, T, N], bf16, tag="mxx")
            for t in range(T):
                pAB = ps_m.tile([128, 2, 512], f32, tag="psm", name="pAB")
                for j in range(3):
                    lhsT = XN[j][: CNT[j], t, :]
                    nc.tensor.matmul(pAB[:, 0, :N], lhsT, M[j][: CNT[j], :],
                                     start=(j == 0), stop=(j == 2))
                    nc.tensor.matmul(pAB[:, 1, :N], lhsT, M2[j][: CNT[j], :],
                                     start=(j == 0), stop=(j == 2))
                if t % 2 == 0:
                    nc.vector.tensor_copy(MXX[:, :, t, :], pAB[:, :, :N])
                else:
                    nc.scalar.activation(MXX[:, :, t, :], pAB[:, :, :N],
                                         AF.Copy)

            if p + 1 < NPAIRS:
                emit_convert(p + 1)
                emit_transposes(p + 1)

            # W stage: out = Xs*Wa + MX*Wb + M2X*Wc + bias, in n-blocks.
            # psum holds (t, n) order; all rhs APs iterate (t, n); the
            # eviction does a strided PSUM read + contiguous SBUF write.
            # Output leaves in 2-block chunks as soon as each chunk's
            # evictions land (chunks 0-1 sync, 2-3 scalar); the last pair
            # goes per-block (alternating rings) to shrink the tail.
            out_hbm = out_ext.ap()[2 * p: 2 * p + 2].rearrange(
                "b c n t -> (b c) (n t)")
            outc = [out_pool.tile([128, 2 * NBLK * T], f32, tag=f"oc{c}",
                                  name=f"oc{c}") for c in range(4)]
            last = p == NPAIRS - 1
            for blk in range(8):
                nb0 = blk * NBLK
                nb = min(NBLK, N - nb0)
                c = blk // 2
                ps = ps_tw.tile([128, T, NBLK], f32, tag="pstw", name="psw")
                pw = ps[:, :, :nb]
                ra = Xs[:, :, nb0: nb0 + nb]
                rb = MXX[:, 0, :, nb0: nb0 + nb]
                rc = MXX[:, 1, :, nb0: nb0 + nb]
                nc.tensor.matmul(pw, Wa[:], ra, start=True, stop=False)
                nc.tensor.matmul(pw, Wb[:], rb, start=False, stop=False)
                nc.tensor.matmul(pw, Wc[:], rc, start=False, stop=True)
                co = (blk % 2) * NBLK * T
                dst = outc[c][:, co: co + nb * T]
                # evict split across both engines so the PSUM bank frees
                # in half the time
                nh = nb // 2
                pr0 = ps[:, :, :nh].rearrange("p t n -> p n t")
                pr1 = ps[:, :, nh:nb].rearrange("p t n -> p n t")
                nc.scalar.activation(dst[:, : nh * T], pr0, AF.Identity,
                                     bias=bias[:, 0:1])
                nc.vector.tensor_scalar_add(dst[:, nh * T:], pr1,
                                            bias[:, 0:1])
                if last:
                    eng = nc.sync if blk % 2 == 0 else nc.scalar
                    eng.dma_start(out_hbm[:, nb0 * T: nb0 * T + nb * T],
                                  outc[c][:, co: co + nb * T])
                elif blk % 2 == 1:
                    c0 = 2 * c * NBLK * T
                    ncols = (blk * NBLK + nb) * T - c0
                    eng = nc.sync if c < 2 else nc.scalar
                    eng.dma_start(out_hbm[:, c0: c0 + ncols],
                                  outc[c][:, :ncols])

    nc.compile()
    return nc


def _get_nc():
    if "nc" not in _cache:
        _cache["nc"] = _build()
    return _cache["nc"]


last_exec_time_ns = None
last_results = None


def kernel(x, adj, W, b):
    from concourse.bass_utils import run_bass_kernel_spmd

    global last_exec_time_ns, last_results
    nc = _get_nc()
    x = np.ascontiguousarray(x, dtype=np.float32)
    adj = np.ascontiguousarray(adj, dtype=np.float32)
    W = np.ascontiguousarray(W, dtype=np.float32)
    b = np.ascontiguousarray(b, dtype=np.float32)
    in_maps = [
        {"x": x[i * B_LOC: (i + 1) * B_LOC], "adj": adj, "W": W, "b": b}
        for i in range(NCORES)
    ]
    trace = bool(os.environ.get("KERNEL_TRACE"))
    res = run_bass_kernel_spmd(nc, in_maps, list(range(NCORES)), trace=trace)
    last_exec_time_ns = res.exec_time_ns
    last_results = res
    out = np.concatenate([res.results[i]["out"] for i in range(NCORES)],
                         axis=0)
    return out


# revision 43
# speedup vs baseline: 1.1659x; 1.1659x over previous
"""AdaptiveGraphConv (Chebyshev K=3) Trainium2 kernel, 8-core data-parallel.

Math (per (batch,time) item, x_item [N,C]):
  M = D^-1/2 A D^-1/2  (normalized adjacency; L = I - M), M symmetric.
  T0 = x; T1 = Lx; T2 = 2L T1 - T0
  out = T0 W0 + T1 W1 + T2 W2 + b
      = x (W0+W1+W2) + (Mx)(-W1-4W2) + (M^2 x)(2W2) + b
M^2 is precomputed once (325x325), so both node-contractions read the same
node-major x and write channel-major results directly (no back-transposes):
  MX_cm[(b,c), i] = sum_j X_nm[j, (b,c)] * M[j, i]   (X_nm as stationary)
Sharding: data-parallel over batch dim B=64 -> 8 batches/core. Laplacian,
weights, bias replicated. No collectives.

Schedule notes (v6):
 - ~18 dense 512-col dummy matmuls at boot keep the PE HAM activity window
   busy (2.4GHz, not the cold 1.2GHz) until the first real matmuls.
 - pair-0 inputs split across both HWDGE rings: sync carries adj + x
   chunks 0/2, the scalar ring carries x chunk 1 (+ W, bias). 128-
   partition (b c)-folded transfers throughout.
 - f32->bf16 convert (strided read reorder (n,t)->(t,n)) split
   vector(c0) / scalar(c1) / vector(c2); PE transposes run in chunk
   order 0,2,1 so the slowest convert (scalar c1) has the most headroom.
 - x loads for pair p+1 issue at the START of pair p (converts+transposes
   emitted between M-apply and W-stage) so the pair boundary never waits
   on DMA.
 - transpose-psum and W-stage-psum share one 4-buf pool: the W-stage gets
   a 4-deep PSUM rotation instead of 2.
 - outputs stream out in 2-block chunks as soon as evicted (sync/scalar
   rings); the last pair goes per-block so the tail is one small DMA.
"""
import os
import sys
import numpy as np

_TRN_REPO = "/opt/trn_rl_repo"
if _TRN_REPO not in sys.path:
    sys.path.insert(0, _TRN_REPO)


def _ensure_ntff_hook():
    """Make antenv.axon_hooks importable so NTFF profiling can register."""
    src = (
        "_hook = None\n"
        "def set_axon_ntff_profile_hook(hook):\n"
        "    global _hook\n"
        "    _hook = hook\n"
        "def get_axon_ntff_profile_hook():\n"
        "    return _hook\n"
    )
    try:
        import antenv  # noqa
        base = os.path.dirname(antenv.__file__)
        path = os.path.join(base, "axon_hooks.py")
        if not os.path.exists(path):
            with open(path, "w") as f:
                f.write(src)
    except Exception:
        pass


_ensure_ntff_hook()

B, C, N, T, K = 64, 64, 325, 12, 3
NCORES = 8
B_LOC = B // NCORES          # 8 batches per core
NPAIRS = B_LOC // 2          # 4 pairs of batches
NT = N * T                   # 3900
CNT = [128, 128, 69]         # node chunk sizes (325 = 128+128+69)
NOFF = [0, 128, 256]
NBLK = 42                    # W-stage node-block (504 cols <= one PSUM bank)

_cache = {}


def _build():
    import concourse.bass as bass  # noqa
    import concourse.bacc as bacc
    import concourse.mybir as mybir
    import concourse.tile as tile
    from concourse import masks
    from contextlib import ExitStack

    f32 = mybir.dt.float32
    bf16 = mybir.dt.bfloat16
    ALU = mybir.AluOpType
    AF = mybir.ActivationFunctionType

    nc = bacc.Bacc("TRN2", target_bir_lowering=False, debug=False,
                   num_devices=NCORES)
    x_ext = nc.dram_tensor("x", [B_LOC, C, N, T], f32, kind="ExternalInput")
    adj_ext = nc.dram_tensor("adj", [N, N], f32, kind="ExternalInput")
    w_ext = nc.dram_tensor("W", [K, C, C], f32, kind="ExternalInput")
    b_ext = nc.dram_tensor("b", [C], f32, kind="ExternalInput")
    out_ext = nc.dram_tensor("out", [B_LOC, C, N, T], f32,
                             kind="ExternalOutput")

    with tile.TileContext(nc) as tc, ExitStack() as ctx:
        const = ctx.enter_context(tc.tile_pool(name="const", bufs=1))
        # M-apply psum: two 2-bank tiles (MX+M2X for one t each, t and
        # t+1 in flight; the single merged eviction overlaps t+1's MMs)
        ps_m = ctx.enter_context(
            tc.tile_pool(name="ps_m", bufs=2, space="PSUM"))
        # shared transpose/W-stage psum rotation: 4 single-bank bufs
        ps_tw = ctx.enter_context(
            tc.tile_pool(name="ps_tw", bufs=4, space="PSUM"))

        xs_pool = ctx.enter_context(tc.tile_pool(name="xs", bufs=2))
        nm_pool = ctx.enter_context(tc.tile_pool(name="nm", bufs=2))
        cm_pool = ctx.enter_context(tc.tile_pool(name="cm", bufs=2))
        out_pool = ctx.enter_context(tc.tile_pool(name="outp", bufs=2))

        state = {}

        def emit_loads(p, split=False):
            # chunked so the per-chunk convert starts as each chunk lands;
            # both batch-halves ride one 128-partition DMA (b,c fold).
            # split=True routes chunk 1 over the scalar ring (pair 0: both
            # rings pull x concurrently).
            Xf = xs_pool.tile([128, N, T], f32, tag="xf", name="xf")
            xsrc = x_ext.ap()[2 * p: 2 * p + 2].rearrange(
                "b c n t -> (b c) n t")
            for i in range(3):
                nsl = slice(NOFF[i], NOFF[i] + CNT[i])
                eng = nc.scalar if (split and i >= 1) else nc.sync
                eng.dma_start(Xf[:, nsl, :], xsrc[:, nsl, :])
            state[p] = Xf

        def emit_convert(p):
            # f32 (n,t) -> bf16 (t,n): the reorder rides on the strided
            # READ (strided reads are cheap; strided writes are not).
            # chunk 0 -> vector, chunk 1 -> split scalar/vector (it's the
            # big slow one), chunk 2 -> vector.
            Xf = state.pop(p)
            Xs = xs_pool.tile([128, T, N], bf16, tag="xsb", name="xsb")
            for i in range(3):
                nsl = slice(NOFF[i], NOFF[i] + CNT[i])
                if i == 1:
                    s0 = Xf[:, nsl, 0:6].rearrange("p n t -> p t n")
                    s1 = Xf[:, nsl, 6:12].rearrange("p n t -> p t n")
                    nc.scalar.activation(Xs[:, 0:6, nsl], s0, AF.Copy)
                    nc.vector.tensor_copy(Xs[:, 6:12, nsl], s1)
                else:
                    srcv = Xf[:, nsl, :].rearrange("p n t -> p t n")
                    nc.vector.tensor_copy(Xs[:, :, nsl], srcv)
            state[p] = Xs

        def emit_transposes(p):
            # node-major X via PE transposes: 3 tiles [n<=128, T, 128],
            # chunk order 0,1,2 matching convert arrival.
            Xs = state[p]
            XN = [nm_pool.tile([128, T, 128], bf16, tag=f"xn{i}",
                               name=f"xn{i}") for i in range(3)]
            for i in range(3):
                nsl = slice(NOFF[i], NOFF[i] + CNT[i])
                for tg in range(2):
                    ps = ps_tw.tile([128, 6, 128], bf16, tag="pstw",
                                    name="pst")
                    for tt in range(6):
                        t = tg * 6 + tt
                        nc.tensor.matmul(
                            ps[: CNT[i], tt, :],
                            Xs[:, t, nsl],
                            idn[:], is_transpose=True)
                    if tg == 0:
                        nc.scalar.activation(
                            XN[i][: CNT[i], 0:6, :],
                            ps[: CNT[i], :, :], AF.Copy)
                    else:
                        nc.vector.tensor_copy(
                            XN[i][: CNT[i], 6:12, :],
                            ps[: CNT[i], :, :])
            state[p] = (Xs, XN)

        # identity + PE warmup: dense 512-col dummy matmuls keep the PE
        # array at high duty so the HAM clock gate opens (2.4GHz) and
        # stays open until the first real matmuls. Results never read.
        idn = const.tile([128, 128], bf16)
        masks.make_identity(nc, idn[:])
        ztile = const.tile([128, 512], bf16)
        nc.vector.memset(ztile[:], 0.0)
        wps = ps_tw.tile([128, 512], f32, tag="pstw", name="wps")
        for _ in range(12):
            nc.tensor.matmul(wps[:, :], idn[:], ztile[:],
                             start=True, stop=True)

        # pair-0 x chunk 0 FIRST on the sync ring (it gates convert ->
        # transposes -> M-apply); adj follows it (its 1300B-row packets
        # dribble at ~90GB/s, and the M/M2 chain has slack until ~17us).
        # Chunks 1/2 pull concurrently on the scalar ring. W and bias
        # ride sync after adj: on the scalar ring their triggers block
        # the scalar FIFO waiting to reuse in-flight x-chunk semaphores.
        emit_loads(0, split=True)
        Af = [const.tile([128, N], f32, tag=f"a{i}", name=f"a{i}")
              for i in range(3)]
        for i in range(3):
            nc.sync.dma_start(Af[i][: CNT[i], :],
                              adj_ext.ap()[NOFF[i]: NOFF[i] + CNT[i], :])
        Wsb = const.tile([128, K, C], f32)
        for h in (0, 1):
            nc.sync.dma_start(Wsb[64 * h: 64 * h + 64, :, :],
                              w_ext.ap().rearrange("k c d -> c k d"))
        bias = const.tile([128, 1], f32)
        for h in (0, 1):
            nc.sync.dma_start(bias[64 * h: 64 * h + 64, :], b_ext.ap())

        # ---- M = D^-1/2 A D^-1/2, three node-row tiles [cnt, 325] bf16 ----
        s_col = [const.tile([128, 1], f32, tag=f"s{i}", name=f"s{i}")
                 for i in range(3)]
        for i in range(3):
            d = const.tile([128, 1], f32, tag="dtmp")
            nc.vector.reduce_sum(d[: CNT[i], :], Af[i][: CNT[i], :],
                                 axis=mybir.AxisListType.X)
            nc.scalar.activation(d[: CNT[i], :], d[: CNT[i], :], AF.Sqrt)
            nc.vector.reciprocal(s_col[i][: CNT[i], :], d[: CNT[i], :])
        # s as a row vector [1, N] via tiny transposes (f32 path)
        idf = const.tile([128, 128], f32)
        masks.make_identity(nc, idf[:])
        ps_s = ps_tw.tile([1, 512], f32, tag="pstw", name="ps_s")
        for i in range(3):
            nc.tensor.matmul(ps_s[0:1, NOFF[i]: NOFF[i] + CNT[i]],
                             s_col[i][: CNT[i], 0:1], idf[: CNT[i], : CNT[i]],
                             is_transpose=True)
        # broadcast s_row to 128 partitions via a bf16 matmul (s only
        # carries bf16 precision anyway: M itself is bf16)
        s_row = const.tile([1, N], bf16)
        nc.vector.tensor_copy(s_row[:], ps_s[0:1, :N])
        ones = const.tile([1, 128], bf16)
        nc.vector.memset(ones[:], 1.0)
        ps_b = ps_tw.tile([128, 512], f32, tag="pstw", name="ps_b")
        nc.tensor.matmul(ps_b[:, :N], ones[0:1, :], s_row[0:1, :])
        s_bc = const.tile([128, N], f32)
        nc.vector.tensor_copy(s_bc[:], ps_b[:, :N])
        # M_i = (s_col * A * s_row) -> bf16
        M = [const.tile([128, N], bf16, tag=f"m{i}", name=f"m{i}")
             for i in range(3)]
        for i in range(3):
            nc.vector.tensor_mul(Af[i][: CNT[i], :], Af[i][: CNT[i], :],
                                 s_bc[: CNT[i], :])
            nc.vector.tensor_scalar_mul(M[i][: CNT[i], :], Af[i][: CNT[i], :],
                                        s_col[i][: CNT[i], 0:1])
        # pair-0 convert AFTER the full M setup: its casts would otherwise
        # block the chain's vector ops (recip, muls) in the vector FIFO
        emit_convert(0)

        # ---- M2 = M @ M, three node-row tiles [cnt, 325] bf16 ----
        M2 = [const.tile([128, N], bf16, tag=f"m2_{i}", name=f"m2_{i}")
              for i in range(3)]
        for j in range(3):
            ps = ps_tw.tile([128, 512], f32, tag="pstw", name="psm2")
            for k in range(3):
                nc.tensor.matmul(
                    ps[: CNT[j], :N],
                    M[k][: CNT[k], NOFF[j]: NOFF[j] + CNT[j]],
                    M[k][: CNT[k], :],
                    start=(k == 0), stop=(k == 2))
            nc.scalar.activation(M2[j][: CNT[j], :], ps[: CNT[j], :N],
                                 AF.Copy)

        # ---- weight combos as block-diagonal [128,128] bf16 (2 copies) ----
        # Wa = W0+W1+W2 ; Wb = -W1-4W2 ; Wc = 2W2 (Wsb/bias DMAs issued
        # at the top, on the sync ring)
        Wa = const.tile([128, 128], bf16)
        Wb = const.tile([128, 128], bf16)
        Wc = const.tile([128, 128], bf16)
        for wt in (Wa, Wb, Wc):
            nc.gpsimd.memset(wt[:], 0.0)
        Wtmp = const.tile([128, C], f32)
        for h in (0, 1):
            r = slice(64 * h, 64 * h + 64)
            # Wa = (W0 + W1) + W2
            nc.vector.tensor_add(Wtmp[r, :], Wsb[r, 0, :], Wsb[r, 1, :])
            nc.vector.tensor_add(Wa[r, r], Wtmp[r, :], Wsb[r, 2, :])
            # Wb = (W2 * -4) - W1
            nc.vector.scalar_tensor_tensor(Wb[r, r], Wsb[r, 2, :], -4.0,
                                           Wsb[r, 1, :], ALU.mult,
                                           ALU.subtract)
            # Wc = 2*W2
            nc.vector.tensor_scalar_mul(Wc[r, r], Wsb[r, 2, :], 2.0)

        emit_transposes(0)
        for p in range(NPAIRS):
            Xs, XN = state.pop(p)

            # next pair's x load issues now so its convert never waits
            if p + 1 < NPAIRS:
                emit_loads(p + 1)

            # M-apply: MX and M2X in one channel-major, t-major tile
            # MXX[:, 0] = MX, MXX[:, 1] = M2X. Both land in one 2-bank
            # psum tile per t and leave in a single eviction.
            MXX = cm_pool.tile([128, 2, T, N], bf16, tag="mxx")
            for t in range(T):
                pAB = ps_m.tile([128, 2, 512], f32, tag="psm", name="pAB")
                for j in range(3):
                    lhsT = XN[j][: CNT[j], t, :]
                    nc.tensor.matmul(pAB[:, 0, :N], lhsT, M[j][: CNT[j], :],
                                     start=(j == 0), stop=(j == 2))
                    nc.tensor.matmul(pAB[:, 1, :N], lhsT, M2[j][: CNT[j], :],
                                     start=(j == 0), stop=(j == 2))
                if t % 2 == 0:
                    nc.vector.tensor_copy(MXX[:, :, t, :], pAB[:, :, :N])
                else:
                    nc.scalar.activation(MXX[:, :, t, :], pAB[:, :, :N],
                                         AF.Copy)

            if p + 1 < NPAIRS:
                emit_convert(p + 1)
                emit_transposes(p + 1)

            # W stage: out = Xs*Wa + MX*Wb + M2X*Wc + bias, in n-blocks.
            # psum holds (t, n) order; all rhs APs iterate (t, n); the
            # eviction does a strided PSUM read + contiguous SBUF write.
            # Output leaves in 2-block chunks as soon as each chunk's
            # evictions land (chunks 0-1 sync, 2-3 scalar); the last pair
            # goes per-block (alternating rings) to shrink the tail.
            out_hbm = out_ext.ap()[2 * p: 2 * p + 2].rearrange(
                "b c n t -> (b c) (n t)")
            outc = [out_pool.tile([128, 2 * NBLK * T], f32, tag=f"oc{c}",
                                  name=f"oc{c}") for c in range(4)]
            last = p == NPAIRS - 1
            for blk in range(8):
                nb0 = blk * NBLK
                nb = min(NBLK, N - nb0)
                c = blk // 2
                ps = ps_tw.tile([128, T, NBLK], f32, tag="pstw", name="psw")
                pw = ps[:, :, :nb]
                ra = Xs[:, :, nb0: nb0 + nb]
                rb = MXX[:, 0, :, nb0: nb0 + nb]
                rc = MXX[:, 1, :, nb0: nb0 + nb]
                nc.tensor.matmul(pw, Wa[:], ra, start=True, stop=False)
                nc.tensor.matmul(pw, Wb[:], rb, start=False, stop=False)
                nc.tensor.matmul(pw, Wc[:], rc, start=False, stop=True)
                co = (blk % 2) * NBLK * T
                dst = outc[c][:, co: co + nb * T]
                # evict split across both engines so the PSUM bank frees
                # in half the time
                nh = nb // 2
                pr0 = ps[:, :, :nh].rearrange("p t n -> p n t")
                pr1 = ps[:, :, nh:nb].rearrange("p t n -> p n t")
                nc.scalar.activation(dst[:, : nh * T], pr0, AF.Identity,
                                     bias=bias[:, 0:1])
                nc.vector.tensor_scalar_add(dst[:, nh * T:], pr1,
                                            bias[:, 0:1])
                if last:
                    eng = nc.sync if blk % 2 == 0 else nc.scalar
                    eng.dma_start(out_hbm[:, nb0 * T: nb0 * T + nb * T],
                                  outc[c][:, co: co + nb * T])
                elif blk % 2 == 1:
                    c0 = 2 * c * NBLK * T
                    ncols = (blk * NBLK + nb) * T - c0
                    eng = nc.sync if c < 2 else nc.scalar
                    eng.dma_start(out_hbm[:, c0: c0 + ncols],
                                  outc[c][:, :ncols])

    nc.compile()
    return nc


def _get_nc():
    if "nc" not in _cache:
        _cache["nc"] = _build()
    return _cache["nc"]


last_exec_time_ns = None
last_results = None


def kernel(x, adj, W, b):
    from concourse.bass_utils import run_bass_kernel_spmd

    global last_exec_time_ns, last_results
    nc = _get_nc()
    x = np.ascontiguousarray(x, dtype=np.float32)
    adj = np.ascontiguousarray(adj, dtype=np.float32)
    W = np.ascontiguousarray(W, dtype=np.float32)
    b = np.ascontiguousarray(b, dtype=np.float32)
    in_maps = [
        {"x": x[i * B_LOC: (i + 1) * B_LOC], "adj": adj, "W": W, "b": b}
        for i in range(NCORES)
    ]
    trace = bool(os.environ.get("KERNEL_TRACE"))
    res = run_bass_kernel_spmd(nc, in_maps, list(range(NCORES)), trace=trace)
    last_exec_time_ns = res.exec_time_ns
    last_results = res
    out = np.concatenate([res.results[i]["out"] for i in range(NCORES)],
                         axis=0)
    return out
